# revision 18
# baseline (speedup 1.0000x reference)
"""DisjointDense (MoE routing) Trainium2 kernel.

out[b] = x[b] @ W[sel[b]] + Bw[sel[b]]   where sel[b] = argmax(one_hot_selector[b])

Strategy: expert-parallel over 8 NeuronCores. Each core owns 8 of the 64
experts, assigned by sorted token count so slot s on every core shares a
per-slot capacity caps[s] (descending — minimal x padding, smallest experts
on the kernel-tail critical path). Host-side sharding routes (sorts) tokens
to their expert's (core, slot); each core runs dense per-slot matmuls
[C_s,256] = [C_s,256]@[256,256] on TensorE; results are gathered back to
original token order on the host, where the bias is added (zero device
cost).

x, W and the output travel in fp16 (halves the HBM/DMA traffic, which is the
binding resource, and runs the PE at 1 cycle/row instead of fp32's 4);
accumulation stays fp32 in PSUM. Load DMAs are sized so the serial HWDGE
descriptor-gen cadence (625ns/DMA + 650ns trigger delay) stays ahead of the
~360B/ns transfer stream; psum->SBUF casts alternate DVE/ACT; output flushes
use separate rings so the tail flush never queues behind a big one.
"""

import sys

for _p in ("/opt/trn_rl_repo",):
    if _p not in sys.path:
        sys.path.append(_p)

import numpy as np

B, D_IN, D_OUT, N_EXP = 4096, 256, 256, 64
N_CORES = 8
E_PC = N_EXP // N_CORES  # experts per core
P = 128  # SBUF partitions / max contraction rows per matmul

_COMPILED = {}  # capacity -> finalized Bass object
_RUNNER = {}  # capacity -> cached jitted SPMD callable
LAST_RESULTS = None  # per-core output dicts of the most recent device run


OUT_GROUPS = ((0, 1, 2, 3), (4, 5), (6, 7))


def _build(caps: tuple):
    """Bass/Tile kernel for one core: 8 expert slots with per-slot capacity.

    Slot s holds one expert per core; caps[s] (sorted descending) is the max
    token count of slot s across cores, so x carries no cross-slot padding.

    Inputs (per core):
      xT  [128, 2*tok] fp16 — gathered tokens, tok = sum(caps); slot s's
          tokens at cols [off_s, off_s+caps[s]) (chunk0) / tok + same
          (chunk1), off_s = sum(caps[:s]).
      Wsh [128, 16*256] fp16 — 8 slots' weights; col block (2s+c)*256 is
          slot s's K-chunk c ([128, 256], rows = d_in c*128..c*128+127).
    Output:
      out [gtok, 256] fp16 — per-group uniform pitch: group g's slots at
          rows gbase_g + i*gcap_g, gcap_g = caps[g[0]] (row-block rearrange
          DMAs need a uniform pitch; host ignores the pad rows).
    """
    import concourse.mybir as mybir
    import concourse.tile as tile
    from concourse import bacc

    f32 = mybir.dt.float32
    f16 = mybir.dt.float16
    tok = sum(caps)
    off = [0]
    for c_ in caps:
        off.append(off[-1] + c_)
    nblks = [-(-c_ // P) for c_ in caps]  # token blocks of <=128 per slot
    gcaps = [caps[g[0]] for g in OUT_GROUPS]
    gtok = sum(len(g) * nblks[g[0]] * (gc if nblks[g[0]] == 1 else P)
               for g, gc in zip(OUT_GROUPS, gcaps))

    nc = bacc.Bacc(None, target_bir_lowering=False)
    xT = nc.dram_tensor("xT", [P, 2 * tok], f16, kind="ExternalInput")
    Wsh = nc.dram_tensor("Wsh", [P, 2 * E_PC * D_OUT], f16, kind="ExternalInput")
    out = nc.dram_tensor("out", [gtok, D_OUT], f16, kind="ExternalOutput")

    with tile.TileContext(nc) as tc:
        with (
            tc.tile_pool(name="xp", bufs=1) as xp,
            tc.tile_pool(name="wp", bufs=7) as wp,
            tc.tile_pool(name="bp", bufs=1) as bp,
            tc.tile_pool(name="op", bufs=2) as op,
            tc.tile_pool(name="pp", bufs=8, space="PSUM") as pp,
        ):
            # Weights for experts 0,1 lead the SP stream (they gate the first
            # expert matmuls); x follows; remaining weights stream behind in
            # pair chunks, then singles for experts 6/7 so the tail
            # dependency is the smallest possible transfer.
            # Weight DMA granules: pair chunks for experts 0-5 (contiguous
            # 1KB runs, one every ~0.73 us), then SINGLE K-chunk granules for
            # experts 6/7 — the tail gating transfers get as small as
            # possible, letting the last matmuls start ~0.2 us earlier.
            wt = {}
            w_granules = [(0, 4), (4, 4), (8, 4), (12, 2), (14, 2)]
            wtiles = {}
            c0, nch = w_granules[0]
            wtile = wp.tile([P, nch * D_OUT], f16, tag="wa")
            nc.sync.dma_start(wtile[:], Wsh[:, c0 * D_OUT : (c0 + nch) * D_OUT])
            wtiles[0] = (c0, nch, wtile)

            # Token activations: one DMA, both K-chunks side by side.
            xtile = xp.tile([P, 2 * tok], f16, tag="x")
            nc.sync.dma_start(xtile[:], xT[:])

            for gi, (c0, nch) in enumerate(w_granules[1:], start=1):
                wtile = wp.tile([P, nch * D_OUT], f16, tag=f"w{gi}")
                nc.sync.dma_start(
                    wtile[:], Wsh[:, c0 * D_OUT : (c0 + nch) * D_OUT]
                )
                wtiles[gi] = (c0, nch, wtile)
            for c0, nch, wtile in wtiles.values():
                for i in range(nch):
                    wt[c0 + i] = wtile[:, i * D_OUT : (i + 1) * D_OUT]

            # PE warm-up: junk matmuls on zeroed tiles while the first DMAs
            # are in flight. ~3.1 us of sustained PE activity releases the
            # HAM clock gate (1.2 -> 2.4 GHz), so the real matmuls run warm.
            wz = bp.tile([1, 512], f32, tag="wz")
            nc.vector.memset(wz[:], 0.0)
            warm_ps = pp.tile([P, 512], f32, tag="ps")
            for wn in (462, 462):
                nc.tensor.matmul(
                    warm_ps[:, 0:wn], wz[:, 0:P], wz[:, 0:wn], start=True, stop=True
                )

            # psum -> SBUF fp16 casts alternate DVE / ACT so neither engine
            # queues (bias is applied host-side during the un-permute).
            ogs = []
            gbase = 0
            gmeta = []
            for grp in OUT_GROUPS:
                nblk = nblks[grp[0]]
                bs = caps[grp[0]] if nblk == 1 else P
                og = op.tile([bs, len(grp) * nblk, D_OUT], f16, tag=f"ot{len(grp)}")
                ogs.append(og)
                gmeta.append((gbase, bs, nblk))
                for gi, s in enumerate(grp):
                    cs = caps[s] if nblk == 1 else P
                    for m in range(nblk):
                        t0 = off[s] + m * cs
                        ps = pp.tile([cs, D_OUT], f32, tag="ps")
                        nc.tensor.matmul(
                            ps[:],
                            xtile[:, t0 : t0 + cs],
                            wt[2 * s],
                            start=True,
                            stop=False,
                        )
                        nc.tensor.matmul(
                            ps[:],
                            xtile[:, tok + t0 : tok + t0 + cs],
                            wt[2 * s + 1],
                            start=False,
                            stop=True,
                        )
                        dst = og[0:cs, gi * nblk + m, :]
                        if (s + m) % 2 == 0:
                            nc.vector.tensor_scalar(
                                dst, ps[:], 0.0, None, op0=mybir.AluOpType.add
                            )
                        else:
                            nc.scalar.activation(
                                dst, ps[:], mybir.ActivationFunctionType.Copy
                            )
                gbase += len(grp) * nblk * bs
            # Flush DMAs after every cast is emitted: the early groups ride
            # the ACT ring (emitted after the last cast, so their SEQ-holds
            # can't delay it), the tail pair rides SP — separate rings, so
            # the tail DMA's descriptor gen never queues behind a big flush.
            for grp, og, (gb, bs, nblk), oeng in zip(
                OUT_GROUPS, ogs, gmeta, (nc.scalar, nc.scalar, nc.sync)
            ):
                oeng.dma_start(
                    out[gb : gb + len(grp) * nblk * bs, :].rearrange(
                        "(blk t) n -> t blk n", t=bs
                    ),
                    og[:],
                )

    nc.compile()
    nc.finalize()
    return nc


def _get_compiled(caps: tuple):
    if caps not in _COMPILED:
        _COMPILED[caps] = _build(caps)
    return _COMPILED[caps]


def _get_runner(caps: tuple):
    """Jit the SPMD dispatch once per cap tuple; reuse across kernel() calls."""
    if caps in _RUNNER:
        return _RUNNER[caps]
    _RUNNER[caps] = _make_runner(_get_compiled(caps))
    return _RUNNER[caps]


def _make_runner(nc):
    """Build a cached jitted SPMD callable for a finalized Bass module.

    Mirrors concourse.bass2jax.run_bass_via_pjrt's multi-core path, but keeps
    the jitted callable so repeat calls skip retracing/recompiling, caches
    device-resident weights, and materializes donated output buffers on
    device.
    """
    import hashlib

    import jax
    import jax.numpy as jnp
    import concourse.mybir as mybir
    from jax.experimental.shard_map import shard_map
    from jax.sharding import Mesh, NamedSharding, PartitionSpec
    from concourse import bass2jax

    bass2jax.install_neuronx_cc_hook()

    partition_name = nc.partition_id_tensor.name if nc.partition_id_tensor else None
    in_names, out_names, out_avals = [], [], []
    for alloc in nc.m.functions[0].allocations:
        if not isinstance(alloc, mybir.MemoryLocationSet):
            continue
        name = alloc.memorylocations[0].name
        if alloc.kind == "ExternalInput":
            if name != partition_name:
                in_names.append(name)
        elif alloc.kind == "ExternalOutput":
            out_names.append(name)
            out_avals.append(
                jax.core.ShapedArray(
                    tuple(alloc.tensor_shape), mybir.dt.np(alloc.dtype)
                )
            )
    n_params = len(in_names)
    all_names = in_names + out_names
    if partition_name is not None:
        all_names = all_names + [partition_name]

    def _body(*args):
        operands = list(args)
        if partition_name is not None:
            operands.append(bass2jax.partition_id_tensor())
        return tuple(
            bass2jax._bass_exec_p.bind(
                *operands,
                out_avals=tuple(out_avals),
                in_names=tuple(all_names),
                out_names=tuple(out_names),
                lowering_input_output_aliases=(),
                sim_require_finite=True,
                sim_require_nnan=True,
                nc=nc,
            )
        )

    devices = jax.devices()[:N_CORES]
    mesh = Mesh(np.asarray(devices), ("core",))
    specs = (PartitionSpec("core"),) * (n_params + len(out_names))
    out_specs = (PartitionSpec("core"),) * len(out_names)
    sharded = jax.jit(
        shard_map(
            _body, mesh=mesh, in_specs=specs, out_specs=out_specs, check_rep=False
        ),
        donate_argnums=tuple(range(n_params, n_params + len(out_names))),
        keep_unused=True,
    )

    core_sh = NamedSharding(mesh, PartitionSpec("core"))
    # Donated output buffers are materialized on-device (their contents are
    # never read — every output byte is written by the kernel), so no zero
    # bytes cross the axon RPC link per call.
    dev_zeros = jax.jit(
        lambda: tuple(
            jnp.zeros((N_CORES * a.shape[0], *a.shape[1:]), a.dtype)
            for a in out_avals
        ),
        out_shardings=(core_sh,) * len(out_avals),
    )
    # Weights/biases rarely change between calls — keep them device-resident
    # keyed by content digest.
    const_cache = {}

    def run(in_maps):
        concat_in = [
            np.ascontiguousarray(
                np.concatenate([m[name] for m in in_maps], axis=0)
            )
            for name in in_names
        ]
        staged = []
        for name, arr in zip(in_names, concat_in):
            if name == "xT":
                staged.append(jax.device_put(arr, core_sh))
                continue
            digest = (name, hashlib.blake2b(arr.tobytes(), digest_size=16).digest())
            if digest not in const_cache:
                if len(const_cache) >= 8:
                    const_cache.pop(next(iter(const_cache)))
                const_cache[digest] = jax.device_put(arr, core_sh)
            staged.append(const_cache[digest])
        out_arrs = sharded(*staged, *dev_zeros())
        return [
            {
                name: np.asarray(out_arrs[i]).reshape(
                    N_CORES, *out_avals[i].shape
                )[c]
                for i, name in enumerate(out_names)
            }
            for c in range(N_CORES)
        ]

    return run


def _dense_fallback(x, one_hot_selector, W, Bw):
    # Only for pathological selectors (not exactly one-hot); never expected.
    v = np.einsum("bi,dio->bdo", x, W)
    h = np.einsum("bd,bdo->bo", one_hot_selector, v)
    return (h + one_hot_selector @ Bw).astype(np.float32)


def kernel(x, one_hot_selector, W, Bw):
    global LAST_RESULTS

    x = np.ascontiguousarray(x, dtype=np.float32)
    one_hot_selector = np.asarray(one_hot_selector, dtype=np.float32)
    W = np.ascontiguousarray(W, dtype=np.float32)
    Bw = np.ascontiguousarray(Bw, dtype=np.float32)

    is_one_hot = (
        one_hot_selector.shape == (x.shape[0], N_EXP)
        and ((one_hot_selector == 0) | (one_hot_selector == 1)).all()
        and (one_hot_selector.sum(axis=1) <= 1).all()
    )
    if not is_one_hot:
        return _dense_fallback(x, one_hot_selector, W, Bw)

    nb = x.shape[0]
    sel = np.argmax(one_hot_selector, axis=1)
    counts = np.bincount(sel, minlength=N_EXP)

    # Slot assignment: sort experts by token count (descending); slot s takes
    # ranks [8s, 8s+8), one per core. caps[s] = max count in slot s,
    # 2-aligned (descending across slots), so x carries minimal padding and
    # the tail slots (last computed, on the kernel's critical path) are the
    # smallest. Multiple of 128 beyond one partition (then uniform).
    ranked = np.argsort(-counts, kind="stable")  # expert ids by count desc
    if counts.max() > P:
        capu = -(-int(counts.max()) // P) * P
        caps = (capu,) * E_PC
    else:
        caps = tuple(
            max(2, -(-int(counts[ranked[s * N_CORES]]) // 2) * 2)
            for s in range(E_PC)
        )
    off = np.concatenate(([0], np.cumsum(caps))).astype(np.int64)
    tok = int(off[-1])
    expert_core = np.empty(N_EXP, dtype=np.int64)
    expert_slot = np.empty(N_EXP, dtype=np.int64)
    for s in range(E_PC):
        for c in range(N_CORES):
            ge = ranked[s * N_CORES + c]
            expert_core[ge] = c
            expert_slot[ge] = s

    # Routing: stable sort by (core, slot), rank within expert -> padded pos.
    key = expert_core[sel] * E_PC + expert_slot[sel]
    order = np.argsort(key, kind="stable")
    kcounts = np.bincount(key, minlength=N_EXP)
    kstarts = np.concatenate(([0], np.cumsum(kcounts)[:-1]))
    rank = np.arange(nb) - kstarts[key[order]]
    ko = key[order]
    xpos = (ko // E_PC) * tok + off[ko % E_PC] + rank

    xpad = np.zeros((N_CORES * tok, D_IN), dtype=np.float16)
    xpad[xpos] = x[order].astype(np.float16)

    # Output positions mirror the kernel's per-group uniform pitch.
    nblks = [-(-c // P) for c in caps]
    slot_out_off = np.zeros(E_PC, dtype=np.int64)
    gb = 0
    for grp in OUT_GROUPS:
        bsz = caps[grp[0]] if nblks[grp[0]] == 1 else P
        for gi, s in enumerate(grp):
            slot_out_off[s] = gb + gi * nblks[grp[0]] * bsz
        gb += len(grp) * nblks[grp[0]] * bsz
    gtok = gb
    opos = (ko // E_PC) * gtok + slot_out_off[ko % E_PC] + rank

    Wf16 = W.astype(np.float16)
    in_maps = []
    for c in range(N_CORES):
        xc = xpad[c * tok : (c + 1) * tok]  # [tok, 256]
        # [128, 2*tok]: cols [0, tok) = d_in 0..127, [tok, 2*tok) = 128..255
        xT2 = np.concatenate([xc[:, :P].T, xc[:, P:].T], axis=1)
        # [128, 16*256]: col block (2s+k)*256 = slot s K-chunk k
        wc = Wf16[ranked[np.arange(E_PC) * N_CORES + c]]  # [8, 256, 256]
        wbig = wc.reshape(E_PC * 2, P, D_OUT).transpose(1, 0, 2).reshape(
            P, 2 * E_PC * D_OUT
        )
        in_maps.append(
            {
                "xT": np.ascontiguousarray(xT2),
                "Wsh": np.ascontiguousarray(wbig),
            }
        )

    run = _get_runner(caps)
    LAST_RESULTS = run(in_maps)
    out_pad = np.concatenate(
        [LAST_RESULTS[c]["out"] for c in range(N_CORES)], axis=0
    )

    y = np.empty((nb, D_OUT), dtype=np.float32)
    y[order] = out_pad[opos].astype(np.float32)
    y += Bw[sel]  # bias applied host-side (zero device cost)
    # Rows whose selector is all-zero produce zero in the reference.
    zero_rows = one_hot_selector.sum(axis=1) == 0
    if zero_rows.any():
        y[zero_rows] = 0.0
    return y


# revision 22
# speedup vs baseline: 1.0079x; 1.0079x over previous
"""DisjointDense (MoE routing) Trainium2 kernel.

out[b] = x[b] @ W[sel[b]] + Bw[sel[b]]   where sel[b] = argmax(one_hot_selector[b])

Strategy: expert-parallel over 8 NeuronCores. Each core owns 8 of the 64
experts, assigned by sorted token count so slot s on every core shares a
per-slot capacity caps[s] (descending — minimal x padding, smallest experts
on the kernel-tail critical path). Host-side sharding routes (sorts) tokens
to their expert's (core, slot); each core runs dense per-slot matmuls
[C_s,256] = [C_s,256]@[256,256] on TensorE; results are gathered back to
original token order on the host, where the bias is added (zero device
cost).

x, W and the output travel in fp16 (halves the HBM/DMA traffic, which is the
binding resource, and runs the PE at 1 cycle/row instead of fp32's 4);
accumulation stays fp32 in PSUM. Load DMAs are sized so the serial HWDGE
descriptor-gen cadence (625ns/DMA + 650ns trigger delay) stays ahead of the
~360B/ns transfer stream; psum->SBUF casts alternate DVE/ACT; output flushes
use separate rings so the tail flush never queues behind a big one.
"""

import sys

for _p in ("/opt/trn_rl_repo",):
    if _p not in sys.path:
        sys.path.append(_p)

import numpy as np

B, D_IN, D_OUT, N_EXP = 4096, 256, 256, 64
N_CORES = 8
E_PC = N_EXP // N_CORES  # experts per core
P = 128  # SBUF partitions / max contraction rows per matmul

_COMPILED = {}  # capacity -> finalized Bass object
_RUNNER = {}  # capacity -> cached jitted SPMD callable
LAST_RESULTS = None  # per-core output dicts of the most recent device run


OUT_GROUPS = ((0, 1, 2), (3, 4, 5), (6, 7))


def _build(caps: tuple):
    """Bass/Tile kernel for one core: 8 expert slots with per-slot capacity.

    Slot s holds one expert per core; caps[s] (sorted descending) is the max
    token count of slot s across cores, so x carries no cross-slot padding.

    Inputs (per core):
      xT  [128, 2*tok] fp16 — gathered tokens, tok = sum(caps); slot s's
          tokens at cols [off_s, off_s+caps[s]) (chunk0) / tok + same
          (chunk1), off_s = sum(caps[:s]).
      Wsh [128, 16*256] fp16 — 8 slots' weights; col block (2s+c)*256 is
          slot s's K-chunk c ([128, 256], rows = d_in c*128..c*128+127).
    Output:
      out [gtok, 256] fp16 — per-group uniform pitch: group g's slots at
          rows gbase_g + i*gcap_g, gcap_g = caps[g[0]] (row-block rearrange
          DMAs need a uniform pitch; host ignores the pad rows).
    """
    import concourse.mybir as mybir
    import concourse.tile as tile
    from concourse import bacc

    f32 = mybir.dt.float32
    f16 = mybir.dt.float16
    tok = sum(caps)
    off = [0]
    for c_ in caps:
        off.append(off[-1] + c_)
    nblks = [-(-c_ // P) for c_ in caps]  # token blocks of <=128 per slot
    gcaps = [caps[g[0]] for g in OUT_GROUPS]
    gtok = sum(len(g) * nblks[g[0]] * (gc if nblks[g[0]] == 1 else P)
               for g, gc in zip(OUT_GROUPS, gcaps))

    nc = bacc.Bacc(None, target_bir_lowering=False)
    xT = nc.dram_tensor("xT", [P, 2 * tok], f16, kind="ExternalInput")
    Wsh = nc.dram_tensor("Wsh", [P, 2 * E_PC * D_OUT], f16, kind="ExternalInput")
    out = nc.dram_tensor("out", [gtok, D_OUT], f16, kind="ExternalOutput")

    with tile.TileContext(nc) as tc:
        with (
            tc.tile_pool(name="xp", bufs=1) as xp,
            tc.tile_pool(name="wp", bufs=7) as wp,
            tc.tile_pool(name="bp", bufs=1) as bp,
            tc.tile_pool(name="op", bufs=2) as op,
            tc.tile_pool(name="pp", bufs=8, space="PSUM") as pp,
        ):
            # Weights for experts 0,1 lead the SP stream (they gate the first
            # expert matmuls); x follows; remaining weights stream behind in
            # pair chunks, then singles for experts 6/7 so the tail
            # dependency is the smallest possible transfer.
            # Weight DMA granules: pair chunks for experts 0-5 (contiguous
            # 1KB runs, one every ~0.73 us), then SINGLE K-chunk granules for
            # experts 6/7 — the tail gating transfers get as small as
            # possible, letting the last matmuls start ~0.2 us earlier.
            wt = {}
            w_granules = [(0, 4), (4, 4), (8, 4), (12, 2), (14, 2)]
            wtiles = {}
            c0, nch = w_granules[0]
            wtile = wp.tile([P, nch * D_OUT], f16, tag="wa")
            nc.sync.dma_start(wtile[:], Wsh[:, c0 * D_OUT : (c0 + nch) * D_OUT])
            wtiles[0] = (c0, nch, wtile)

            # Token activations: one DMA, both K-chunks side by side.
            xtile = xp.tile([P, 2 * tok], f16, tag="x")
            nc.sync.dma_start(xtile[:], xT[:])

            for gi, (c0, nch) in enumerate(w_granules[1:], start=1):
                wtile = wp.tile([P, nch * D_OUT], f16, tag=f"w{gi}")
                nc.sync.dma_start(
                    wtile[:], Wsh[:, c0 * D_OUT : (c0 + nch) * D_OUT]
                )
                wtiles[gi] = (c0, nch, wtile)
            for c0, nch, wtile in wtiles.values():
                for i in range(nch):
                    wt[c0 + i] = wtile[:, i * D_OUT : (i + 1) * D_OUT]

            # PE warm-up: junk matmuls on zeroed tiles while the first DMAs
            # are in flight. ~3.1 us of sustained PE activity releases the
            # HAM clock gate (1.2 -> 2.4 GHz), so the real matmuls run warm.
            wz = bp.tile([1, 512], f32, tag="wz")
            nc.vector.memset(wz[:], 0.0)
            warm_ps = pp.tile([P, 512], f32, tag="ps")
            for wn in (462, 462):
                nc.tensor.matmul(
                    warm_ps[:, 0:wn], wz[:, 0:P], wz[:, 0:wn], start=True, stop=True
                )

            # psum -> SBUF fp16 casts alternate DVE / ACT so neither engine
            # queues (bias is applied host-side during the un-permute).
            ogs = []
            gbase = 0
            gmeta = []
            for grp in OUT_GROUPS:
                nblk = nblks[grp[0]]
                bs = caps[grp[0]] if nblk == 1 else P
                og = op.tile([bs, len(grp) * nblk, D_OUT], f16, tag=f"ot{len(grp)}")
                ogs.append(og)
                gmeta.append((gbase, bs, nblk))
                for gi, s in enumerate(grp):
                    cs = caps[s] if nblk == 1 else P
                    for m in range(nblk):
                        t0 = off[s] + m * cs
                        ps = pp.tile([cs, D_OUT], f32, tag="ps")
                        nc.tensor.matmul(
                            ps[:],
                            xtile[:, t0 : t0 + cs],
                            wt[2 * s],
                            start=True,
                            stop=False,
                        )
                        nc.tensor.matmul(
                            ps[:],
                            xtile[:, tok + t0 : tok + t0 + cs],
                            wt[2 * s + 1],
                            start=False,
                            stop=True,
                        )
                        dst = og[0:cs, gi * nblk + m, :]
                        if (s + m) % 2 == 0:
                            nc.vector.tensor_scalar(
                                dst, ps[:], 0.0, None, op0=mybir.AluOpType.add
                            )
                        else:
                            nc.scalar.activation(
                                dst, ps[:], mybir.ActivationFunctionType.Copy
                            )
                gbase += len(grp) * nblk * bs
            # Flush DMAs after every cast is emitted: the early groups ride
            # the ACT ring (emitted after the last cast, so their SEQ-holds
            # can't delay it), the tail pair rides SP — separate rings, so
            # the tail DMA's descriptor gen never queues behind a big flush.
            for grp, og, (gb, bs, nblk), oeng in zip(
                OUT_GROUPS, ogs, gmeta, (nc.scalar, nc.scalar, nc.sync)
            ):
                oeng.dma_start(
                    out[gb : gb + len(grp) * nblk * bs, :].rearrange(
                        "(blk t) n -> t blk n", t=bs
                    ),
                    og[:],
                )

    nc.compile()
    nc.finalize()
    return nc


def _get_compiled(caps: tuple):
    if caps not in _COMPILED:
        _COMPILED[caps] = _build(caps)
    return _COMPILED[caps]


def _get_runner(caps: tuple):
    """Jit the SPMD dispatch once per cap tuple; reuse across kernel() calls."""
    if caps in _RUNNER:
        return _RUNNER[caps]
    _RUNNER[caps] = _make_runner(_get_compiled(caps))
    return _RUNNER[caps]


def _make_runner(nc):
    """Build a cached jitted SPMD callable for a finalized Bass module.

    Mirrors concourse.bass2jax.run_bass_via_pjrt's multi-core path, but keeps
    the jitted callable so repeat calls skip retracing/recompiling, caches
    device-resident weights, and materializes donated output buffers on
    device.
    """
    import hashlib

    import jax
    import jax.numpy as jnp
    import concourse.mybir as mybir
    from jax.experimental.shard_map import shard_map
    from jax.sharding import Mesh, NamedSharding, PartitionSpec
    from concourse import bass2jax

    bass2jax.install_neuronx_cc_hook()

    partition_name = nc.partition_id_tensor.name if nc.partition_id_tensor else None
    in_names, out_names, out_avals = [], [], []
    for alloc in nc.m.functions[0].allocations:
        if not isinstance(alloc, mybir.MemoryLocationSet):
            continue
        name = alloc.memorylocations[0].name
        if alloc.kind == "ExternalInput":
            if name != partition_name:
                in_names.append(name)
        elif alloc.kind == "ExternalOutput":
            out_names.append(name)
            out_avals.append(
                jax.core.ShapedArray(
                    tuple(alloc.tensor_shape), mybir.dt.np(alloc.dtype)
                )
            )
    n_params = len(in_names)
    all_names = in_names + out_names
    if partition_name is not None:
        all_names = all_names + [partition_name]

    def _body(*args):
        operands = list(args)
        if partition_name is not None:
            operands.append(bass2jax.partition_id_tensor())
        return tuple(
            bass2jax._bass_exec_p.bind(
                *operands,
                out_avals=tuple(out_avals),
                in_names=tuple(all_names),
                out_names=tuple(out_names),
                lowering_input_output_aliases=(),
                sim_require_finite=True,
                sim_require_nnan=True,
                nc=nc,
            )
        )

    devices = jax.devices()[:N_CORES]
    mesh = Mesh(np.asarray(devices), ("core",))
    specs = (PartitionSpec("core"),) * (n_params + len(out_names))
    out_specs = (PartitionSpec("core"),) * len(out_names)
    sharded = jax.jit(
        shard_map(
            _body, mesh=mesh, in_specs=specs, out_specs=out_specs, check_rep=False
        ),
        donate_argnums=tuple(range(n_params, n_params + len(out_names))),
        keep_unused=True,
    )

    core_sh = NamedSharding(mesh, PartitionSpec("core"))
    # Donated output buffers are materialized on-device (their contents are
    # never read — every output byte is written by the kernel), so no zero
    # bytes cross the axon RPC link per call.
    dev_zeros = jax.jit(
        lambda: tuple(
            jnp.zeros((N_CORES * a.shape[0], *a.shape[1:]), a.dtype)
            for a in out_avals
        ),
        out_shardings=(core_sh,) * len(out_avals),
    )
    # Weights/biases rarely change between calls — keep them device-resident
    # keyed by content digest.
    const_cache = {}

    def run(in_maps):
        concat_in = [
            np.ascontiguousarray(
                np.concatenate([m[name] for m in in_maps], axis=0)
            )
            for name in in_names
        ]
        staged = []
        for name, arr in zip(in_names, concat_in):
            if name == "xT":
                staged.append(jax.device_put(arr, core_sh))
                continue
            digest = (name, hashlib.blake2b(arr.tobytes(), digest_size=16).digest())
            if digest not in const_cache:
                if len(const_cache) >= 8:
                    const_cache.pop(next(iter(const_cache)))
                const_cache[digest] = jax.device_put(arr, core_sh)
            staged.append(const_cache[digest])
        out_arrs = sharded(*staged, *dev_zeros())
        return [
            {
                name: np.asarray(out_arrs[i]).reshape(
                    N_CORES, *out_avals[i].shape
                )[c]
                for i, name in enumerate(out_names)
            }
            for c in range(N_CORES)
        ]

    return run


def _dense_fallback(x, one_hot_selector, W, Bw):
    # Only for pathological selectors (not exactly one-hot); never expected.
    v = np.einsum("bi,dio->bdo", x, W)
    h = np.einsum("bd,bdo->bo", one_hot_selector, v)
    return (h + one_hot_selector @ Bw).astype(np.float32)


def kernel(x, one_hot_selector, W, Bw):
    global LAST_RESULTS

    x = np.ascontiguousarray(x, dtype=np.float32)
    one_hot_selector = np.asarray(one_hot_selector, dtype=np.float32)
    W = np.ascontiguousarray(W, dtype=np.float32)
    Bw = np.ascontiguousarray(Bw, dtype=np.float32)

    is_one_hot = (
        one_hot_selector.shape == (x.shape[0], N_EXP)
        and ((one_hot_selector == 0) | (one_hot_selector == 1)).all()
        and (one_hot_selector.sum(axis=1) <= 1).all()
    )
    if not is_one_hot:
        return _dense_fallback(x, one_hot_selector, W, Bw)

    nb = x.shape[0]
    sel = np.argmax(one_hot_selector, axis=1)
    counts = np.bincount(sel, minlength=N_EXP)

    # Slot assignment: sort experts by token count (descending); slot s takes
    # ranks [8s, 8s+8), one per core. caps[s] = max count in slot s,
    # 2-aligned (descending across slots), so x carries minimal padding and
    # the tail slots (last computed, on the kernel's critical path) are the
    # smallest. Multiple of 128 beyond one partition (then uniform).
    ranked = np.argsort(-counts, kind="stable")  # expert ids by count desc
    if counts.max() > P:
        capu = -(-int(counts.max()) // P) * P
        caps = (capu,) * E_PC
    else:
        caps = tuple(
            max(2, -(-int(counts[ranked[s * N_CORES]]) // 2) * 2)
            for s in range(E_PC)
        )
    off = np.concatenate(([0], np.cumsum(caps))).astype(np.int64)
    tok = int(off[-1])
    expert_core = np.empty(N_EXP, dtype=np.int64)
    expert_slot = np.empty(N_EXP, dtype=np.int64)
    for s in range(E_PC):
        for c in range(N_CORES):
            ge = ranked[s * N_CORES + c]
            expert_core[ge] = c
            expert_slot[ge] = s

    # Routing: stable sort by (core, slot), rank within expert -> padded pos.
    key = expert_core[sel] * E_PC + expert_slot[sel]
    order = np.argsort(key, kind="stable")
    kcounts = np.bincount(key, minlength=N_EXP)
    kstarts = np.concatenate(([0], np.cumsum(kcounts)[:-1]))
    rank = np.arange(nb) - kstarts[key[order]]
    ko = key[order]
    xpos = (ko // E_PC) * tok + off[ko % E_PC] + rank

    xpad = np.zeros((N_CORES * tok, D_IN), dtype=np.float16)
    xpad[xpos] = x[order].astype(np.float16)

    # Output positions mirror the kernel's per-group uniform pitch.
    nblks = [-(-c // P) for c in caps]
    slot_out_off = np.zeros(E_PC, dtype=np.int64)
    gb = 0
    for grp in OUT_GROUPS:
        bsz = caps[grp[0]] if nblks[grp[0]] == 1 else P
        for gi, s in enumerate(grp):
            slot_out_off[s] = gb + gi * nblks[grp[0]] * bsz
        gb += len(grp) * nblks[grp[0]] * bsz
    gtok = gb
    opos = (ko // E_PC) * gtok + slot_out_off[ko % E_PC] + rank

    Wf16 = W.astype(np.float16)
    in_maps = []
    for c in range(N_CORES):
        xc = xpad[c * tok : (c + 1) * tok]  # [tok, 256]
        # [128, 2*tok]: cols [0, tok) = d_in 0..127, [tok, 2*tok) = 128..255
        xT2 = np.concatenate([xc[:, :P].T, xc[:, P:].T], axis=1)
        # [128, 16*256]: col block (2s+k)*256 = slot s K-chunk k
        wc = Wf16[ranked[np.arange(E_PC) * N_CORES + c]]  # [8, 256, 256]
        wbig = wc.reshape(E_PC * 2, P, D_OUT).transpose(1, 0, 2).reshape(
            P, 2 * E_PC * D_OUT
        )
        in_maps.append(
            {
                "xT": np.ascontiguousarray(xT2),
                "Wsh": np.ascontiguousarray(wbig),
            }
        )

    run = _get_runner(caps)
    LAST_RESULTS = run(in_maps)
    out_pad = np.concatenate(
        [LAST_RESULTS[c]["out"] for c in range(N_CORES)], axis=0
    )

    y = np.empty((nb, D_OUT), dtype=np.float32)
    y[order] = out_pad[opos].astype(np.float32)
    y += Bw[sel]  # bias applied host-side (zero device cost)
    # Rows whose selector is all-zero produce zero in the reference.
    zero_rows = one_hot_selector.sum(axis=1) == 0
    if zero_rows.any():
        y[zero_rows] = 0.0
    return y
